# revision 1
# baseline (speedup 1.0000x reference)
"""GCNII layer on 8 TRN2 NeuronCores (Bass/Tile).

Strategy: partition nodes (and their incoming edges, bucketed by dst) across
the 8 cores; replicate the feature table (bf16) in every core's DRAM.  Per
core, nodes are greedily packed into chunks of 128 output slots balancing the
per-(chunk, src-subrange) edge counts so each (chunk,sub) fits TR 128-edge
tiles.  The feature table is split into 4 sub-tables of <32768 rows so
dma_gather's int16 indices reach every row; gathers run one call per
(4-chunk group, subrange) — the 4 calls of a group run on the 4 SWDGE queues
concurrently to parallelize Q7 descriptor generation (the end-to-end pacer).
The per-tile one-hot selection matrices are precomputed on the host as 0/1
fp8 planes (integer data) and DMAed in (fp8 x bf16 matmul is exact for 0/1
weights); the folded edge weight 0.9*rsqrt(deg[src])*rsqrt(deg[dst]) is
computed on device and applied to the gathered rows in-place with one wide
broadcast tensor_tensor per gather call.  TensorE matmuls accumulate the
edge-normalized neighbor sum in fp32 PSUM (weights stream at 56ns/matmul);
the PSUM result is already the blended pre-residual value, so the epilogue
is wide [128,512] 4-chunk blocks: PSUM copy, a transpose via PE folding the
0.1*I initial residual, and the identity-mapped W matmul with fused ReLU,
written out transposed ([D, SLOTS]) and un-permuted on the host.  Host-side
work is integer bucketing/layout; all per-edge float math runs on device.
"""

import sys

if "/opt/trn_rl_repo" not in sys.path:
    sys.path.insert(0, "/opt/trn_rl_repo")

from contextlib import ExitStack

import ml_dtypes
import numpy as np

N, E, D, NC = 100000, 1600000, 128, 8
NPC = N // NC            # nodes per core: 12500
ALPHA, BETA = 0.1, 0.5
NSUB = 4                 # feature-table subranges (int16 index limit)
SR = 25000               # rows per subrange
BLK = 4                  # chunks per wide epilogue block (psum 512 free dim)

F32 = np.float32
BF16 = ml_dtypes.bfloat16
FP8 = ml_dtypes.float8_e4m3


def _wrap_idx(seq):
    """dma_gather index layout: i -> [i % 16, i // 16], replicated to 128
    partitions (one copy per Q7 core)."""
    blk = seq.reshape(-1, 16).T
    return np.tile(blk, (8, 1))


def _balance_nodes(deg_sub, chunks):
    """Greedy 4-dim balancing: assign nodes (rows of deg_sub [n,4]) to
    `chunks` bins (<=128 nodes each) minimizing max per-(bin,sub) load."""
    n = deg_sub.shape[0]
    order = np.argsort(-deg_sub.sum(1), kind="stable")
    loads = np.zeros((chunks, NSUB), np.int64)
    counts = np.zeros(chunks, np.int64)
    chunk_of = np.empty(n, np.int64)
    slot_of = np.empty(n, np.int64)
    for i in order:
        score = np.max(loads + deg_sub[i], axis=1)
        score[counts >= 128] = 1 << 60
        c = int(np.argmin(score))
        chunk_of[i] = c
        slot_of[i] = counts[c]
        counts[c] += 1
        loads[c] += deg_sub[i]
    return chunk_of, slot_of, loads


def _host_prep(features, initial_features, W, src, dst):
    """Integer bucketing/layout prep -> per-core device arrays."""
    src = np.ascontiguousarray(src).astype(np.int64, copy=False)
    dst = np.ascontiguousarray(dst).astype(np.int64, copy=False)
    deg = np.bincount(dst, minlength=N)
    degc = np.maximum(deg, 1).astype(np.int64)
    core_of = dst // NPC

    CHUNKS = 104
    cores_tmp = []
    max_load = 0
    for c in range(NC):
        em = core_of == c
        e_src = src[em]
        e_loc = dst[em] - c * NPC
        e_sub = e_src // SR
        deg_sub = np.zeros((NPC, NSUB), np.int64)
        np.add.at(deg_sub, (e_loc, e_sub), 1)
        chunk_of, slot_of, loads = _balance_nodes(deg_sub, CHUNKS)
        max_load = max(max_load, int(loads.max()))
        cores_tmp.append((e_src, e_loc, e_sub, chunk_of, slot_of))
    TR = (max_load + 127) // 128         # tiles per (chunk, subrange)
    TT = NSUB * TR                       # tiles per chunk
    cap = TR * 128
    SLOTS = CHUNKS * 128
    COLS = CHUNKS * TT
    G = 4                                # chunks per gather group
    NG = CHUNKS // G
    per_core = []
    for c in range(NC):
        e_src, e_loc, e_sub, chunk_of, slot_of = cores_tmp[c]
        e_chunk = chunk_of[e_loc]
        e_slot = slot_of[e_loc]
        o = np.lexsort((e_src, e_sub, e_chunk))
        e_src, e_slot, e_chunk, e_sub = e_src[o], e_slot[o], e_chunk[o], e_sub[o]
        e_ddeg = degc[e_loc[o] + c * NPC]    # deg[dst] per (sorted) edge
        e_sdeg = degc[e_src]
        cnt = np.bincount(e_chunk * NSUB + e_sub, minlength=CHUNKS * NSUB)
        # [CHUNKS, NSUB, cap] per-(chunk,subrange) padded segments
        idx_arr = np.zeros((CHUNKS, NSUB, cap), np.int16)
        rel_arr = np.full((CHUNKS, NSUB, cap), -1, np.int64)
        dsd_arr = np.ones((CHUNKS, NSUB, cap), F32)   # deg[src]*deg[dst]
        starts = np.zeros(CHUNKS * NSUB, np.int64)
        starts[1:] = np.cumsum(cnt)[:-1]
        pos = np.arange(len(e_src)) - starts[e_chunk * NSUB + e_sub]
        idx_arr[e_chunk, e_sub, pos] = (e_src - e_sub * SR).astype(np.int16)
        rel_arr[e_chunk, e_sub, pos] = e_slot
        dsd_arr[e_chunk, e_sub, pos] = (e_sdeg * e_ddeg).astype(F32)
        # 0/1 one-hot planes, fp8: row p, col (c*TT + k)*128 + slot
        rel4 = rel_arr.reshape(CHUNKS, NSUB, TR, 128)      # (c, r, t, p)
        ci, ri, ti, pi = np.nonzero(rel4 >= 0)
        si = rel4[ci, ri, ti, pi]
        colk = (ci * TT + ri * TR + ti) * 128 + si
        oh8 = np.zeros((128, COLS * 128), FP8)
        oh8[pi, colk] = 1.0
        # gather-call order: (group g, sub r, chunk-local, tile t, part p)
        idx_g = idx_arr.reshape(NG, G, NSUB, cap)
        idx_flat = idx_g.transpose(0, 2, 1, 3).reshape(-1)
        idx_dev = _wrap_idx(idx_flat).astype(np.int16)   # [128, COLS*8]
        dsd_g = dsd_arr.reshape(NG, G, NSUB, cap).transpose(0, 2, 1, 3)
        dsd_dev = np.ascontiguousarray(dsd_g.reshape(COLS, 128).T)

        # node layout tables
        nodelist = np.full((CHUNKS, 128), -1, np.int64)
        nodelist[chunk_of, slot_of] = np.arange(NPC)
        glob = np.where(nodelist >= 0, nodelist + c * NPC, -1)
        init_perm = np.zeros((SLOTS, D), F32)
        gv = glob.reshape(-1)
        init_perm[gv >= 0] = initial_features[gv[gv >= 0]]
        per_core.append(
            dict(
                eidx=np.ascontiguousarray(idx_dev),
                oh8=oh8,
                edsd=dsd_dev,
                initp=init_perm,
                glob=glob,
            )
        )
    return per_core, TR, CHUNKS, G


_BUILD_CACHE = {}


def _build(TR, chunks, G, n_rows=N, nsub=NSUB, sr=SR):
    key = (TR, chunks, G, n_rows, nsub, sr)
    if key in _BUILD_CACHE:
        return _BUILD_CACHE[key]
    import concourse.bacc as bacc
    import concourse.bass as bass  # noqa: F401
    import concourse.mybir as mybir
    import concourse.tile as tile

    f32 = mybir.dt.float32
    bf16 = mybir.dt.bfloat16
    fp8 = mybir.dt.float8e4
    i16 = mybir.dt.int16
    Alu = mybir.AluOpType
    Act = mybir.ActivationFunctionType

    TT = nsub * TR
    SLOTS_ = chunks * 128
    COLS = chunks * TT               # total edge-tile columns
    IDXC = COLS * 8                  # idx cols (int16, 16-wrap => /16*128)
    NG = chunks // G                 # gather groups
    NIC = G * TR * 128               # idxs per gather call
    ICC = NIC // 16                  # idx cols per gather call
    GTR = G * TR                     # tiles per gather call

    nc = bacc.Bacc("TRN2", target_bir_lowering=False, num_swdge_queues=4)
    feats = nc.dram_tensor("feats", [n_rows, D], bf16, kind="ExternalInput")
    wt = nc.dram_tensor("wt", [D, D], f32, kind="ExternalInput")
    ident = nc.dram_tensor("ident", [128, 128], f32, kind="ExternalInput")
    ident01 = nc.dram_tensor("ident01", [128, 128], f32, kind="ExternalInput")
    eidx = nc.dram_tensor("eidx", [128, IDXC], i16, kind="ExternalInput")
    oh8 = nc.dram_tensor("oh8", [128, COLS * 128], fp8, kind="ExternalInput")
    edsd = nc.dram_tensor("edsd", [128, COLS], f32, kind="ExternalInput")
    initp = nc.dram_tensor("initp", [SLOTS_, D], f32, kind="ExternalInput")
    outT = nc.dram_tensor("outT", [D, SLOTS_], f32, kind="ExternalOutput")

    with tile.TileContext(nc) as tc, ExitStack() as ctx:
        const = ctx.enter_context(tc.tile_pool(name="const", bufs=1))
        gpools = [ctx.enter_context(tc.tile_pool(name=f"g{r}", bufs=5))
                  for r in range(nsub)]
        oh8pool = ctx.enter_context(tc.tile_pool(name="oh8p", bufs=6))
        epool = ctx.enter_context(tc.tile_pool(name="ep", bufs=3))
        ipool = ctx.enter_context(tc.tile_pool(name="init", bufs=6))
        opool = ctx.enter_context(tc.tile_pool(name="ob", bufs=3))
        ps_agg = ctx.enter_context(tc.tile_pool(name="psagg", bufs=3, space="PSUM"))
        ps_tr = ctx.enter_context(tc.tile_pool(name="pstr", bufs=2, space="PSUM"))
        ps_mm = ctx.enter_context(tc.tile_pool(name="psmm", bufs=2, space="PSUM"))

        wt_sb = const.tile([128, 128], f32)
        nc.sync.dma_start(out=wt_sb[:], in_=wt[:])
        id_sb = const.tile([128, 128], f32)
        nc.sync.dma_start(out=id_sb[:], in_=ident[:])
        id01_sb = const.tile([128, 128], f32)
        nc.sync.dma_start(out=id01_sb[:], in_=ident01[:])
        idx_sb = const.tile([128, IDXC], i16)
        nc.sync.dma_start(out=idx_sb[:], in_=eidx[:])
        dsd_sb = const.tile([128, COLS], f32)
        nc.sync.dma_start(out=dsd_sb[:], in_=edsd[:])

        # scl = 0.9 * rsqrt(deg[src]*deg[dst]) per edge, gather-call order
        sclf_sb = const.tile([128, COLS], f32)
        nc.scalar.activation(sclf_sb[:], dsd_sb[:], Act.Sqrt)
        nc.vector.reciprocal(sclf_sb[:], sclf_sb[:])
        nc.vector.tensor_scalar(sclf_sb[:], sclf_sb[:], 1.0 - ALPHA, None,
                                Alu.mult)
        scl_sb = const.tile([128, COLS], bf16)
        nc.scalar.activation(scl_sb[:], sclf_sb[:], Act.Copy)

        call = 0
        for g in range(NG):
            bufs = []
            for r in range(nsub):
                lo = r * sr
                hi = min(n_rows, (r + 1) * sr)
                bufr = gpools[r].tile([128, NIC], bf16, name=f"b{r}")
                nc.gpsimd.dma_gather(
                    out_ap=bufr[:].rearrange("p (t d) -> p t d", t=GTR),
                    in_ap=feats[lo:hi, :],
                    idxs_ap=idx_sb[:, call * ICC:(call + 1) * ICC],
                    num_idxs=NIC,
                    num_idxs_reg=NIC,
                    elem_size=D,
                    single_packet=False,
                    queue_num=call % 4,
                )
                # fold the edge weight into the gathered rows (in place)
                nc.vector.tensor_tensor(
                    bufr[:].rearrange("p (t d) -> p t d", t=GTR),
                    bufr[:].rearrange("p (t d) -> p t d", t=GTR),
                    scl_sb[:, call * GTR:(call + 1) * GTR]
                    .unsqueeze(-1).broadcast_to([128, GTR, 128]),
                    Alu.mult)
                bufs.append(bufr)
                call += 1
            for cl in range(G):
                c = g * G + cl
                bi, bc = divmod(c, BLK)
                if bc == 0:
                    psw = ps_agg.tile([128, BLK * 128], f32, space="PSUM",
                                      name="psw")
                oh = oh8pool.tile([128, TT * 128], fp8, name="oh")
                nc.sync.dma_start(
                    out=oh[:], in_=oh8[:, c * TT * 128:(c + 1) * TT * 128])
                for k in range(TT):
                    r, t = divmod(k, TR)
                    off = (cl * TR + t) * 128
                    nc.tensor.matmul(
                        psw[:, bc * 128:(bc + 1) * 128],
                        lhsT=oh[:, k * 128:(k + 1) * 128],
                        rhs=bufs[r][:, off:off + 128],
                        start=(k == 0),
                        stop=(k == TT - 1),
                    )
                if bc == BLK - 1:
                    # wide epilogue for chunks [bi*BLK, (bi+1)*BLK)
                    h2w = epool.tile([128, BLK * 128], f32, tag="h2w")
                    nc.scalar.activation(h2w[:], psw[:], Act.Copy)
                    ptrw = ps_tr.tile([128, BLK * 128], f32, space="PSUM",
                                      name="ptrw")
                    for j in range(BLK):
                        cj = bi * BLK + j
                        itile = ipool.tile([128, 128], f32, name="itile")
                        nc.sync.dma_start(
                            out=itile[:],
                            in_=initp[cj * 128:(cj + 1) * 128, :])
                        nc.tensor.matmul(
                            ptrw[:, j * 128:(j + 1) * 128],
                            lhsT=h2w[:, j * 128:(j + 1) * 128], rhs=id_sb[:],
                            start=True, stop=False)
                        nc.tensor.matmul(
                            ptrw[:, j * 128:(j + 1) * 128],
                            lhsT=itile[:], rhs=id01_sb[:],
                            start=False, stop=True)
                    h3tw = epool.tile([128, BLK * 128], f32, tag="h3tw")
                    nc.scalar.activation(h3tw[:], ptrw[:], Act.Copy)
                    pmmw = ps_mm.tile([128, BLK * 128], f32, space="PSUM",
                                      name="pmmw")
                    nc.tensor.matmul(pmmw[:], lhsT=wt_sb[:], rhs=h3tw[:],
                                     start=True, stop=False)
                    nc.tensor.matmul(pmmw[:], lhsT=id_sb[:], rhs=h3tw[:],
                                     start=False, stop=True)
                    obw = opool.tile([128, BLK * 128], f32)
                    nc.scalar.activation(obw[:], pmmw[:], Act.Relu, scale=BETA)
                    nc.sync.dma_start(
                        out=outT[:, bi * BLK * 128:(bi + 1) * BLK * 128],
                        in_=obw[:])

    nc.compile()
    _BUILD_CACHE[key] = nc
    return nc


def _install_ntff_shim():
    """antenv.axon_hooks is absent in this image; shim it and wire the real
    NTFF profiling hook via ctypes so trace=True works under axon."""
    import contextlib
    import ctypes
    import types

    try:
        from antenv import axon_hooks  # noqa: F401
        return
    except ImportError:
        pass
    import antenv

    mod = types.ModuleType("antenv.axon_hooks")
    _hook = [None]
    mod.set_axon_ntff_profile_hook = lambda h: _hook.__setitem__(0, h)
    mod.get_axon_ntff_profile_hook = lambda: _hook[0]
    sys.modules["antenv.axon_hooks"] = mod
    antenv.axon_hooks = mod
    try:
        lib = ctypes.CDLL("/opt/axon/libaxon_pjrt.so")
    except OSError:
        return
    if not hasattr(lib, "axon_start_nrt_profile"):
        return
    lib.axon_start_nrt_profile.argtypes = [
        ctypes.POINTER(ctypes.c_int64),
        ctypes.c_size_t,
    ]
    lib.axon_start_nrt_profile.restype = ctypes.c_int64
    lib.axon_stop_nrt_profile.argtypes = [ctypes.c_char_p]
    lib.axon_stop_nrt_profile.restype = ctypes.c_int64

    @contextlib.contextmanager
    def _hook_cm(output_dir, device_ids):
        import jax

        jax.devices()
        if device_ids:
            ids = (ctypes.c_int64 * len(device_ids))(*device_ids)
            rc = lib.axon_start_nrt_profile(ids, len(device_ids))
        else:
            rc = lib.axon_start_nrt_profile(None, 0)
        if rc != 0:
            raise RuntimeError(f"axon_start_nrt_profile rc={rc}")
        try:
            yield
        finally:
            rc = lib.axon_stop_nrt_profile(output_dir.encode())
            if rc != 0:
                print(f"WARNING: axon_stop_nrt_profile rc={rc}", flush=True)

    mod.set_axon_ntff_profile_hook(_hook_cm)


def _run(inputs, trace=False, trace_cores=None):
    from concourse import bass_utils

    if trace:
        _install_ntff_shim()
    features = np.ascontiguousarray(np.asarray(inputs["features"], dtype=F32))
    initial_features = np.ascontiguousarray(
        np.asarray(inputs["initial_features"], dtype=F32)
    )
    W = np.asarray(inputs["W"], dtype=F32)
    src = np.asarray(inputs["src"])
    dst = np.asarray(inputs["dst"])
    per_core, TR, CHUNKS, G = _host_prep(features, initial_features, W, src, dst)
    nc = _build(TR, CHUNKS, G)
    feats_bf = np.ascontiguousarray(features.astype(BF16))
    wt_np = np.ascontiguousarray(W.T)
    ident_np = np.eye(128, dtype=F32)
    ident01_np = np.eye(128, dtype=F32) * ALPHA
    in_maps = []
    for c in range(NC):
        pc = per_core[c]
        in_maps.append(
            dict(
                feats=feats_bf,
                wt=wt_np,
                ident=ident_np,
                ident01=ident01_np,
                eidx=pc["eidx"],
                oh8=pc["oh8"],
                edsd=pc["edsd"],
                initp=pc["initp"],
            )
        )
    res = bass_utils.run_bass_kernel_spmd(
        nc,
        in_maps,
        core_ids=list(range(NC)),
        trace=trace,
        trace_cores=trace_cores,
    )
    result = np.empty((N, D), F32)
    for c in range(NC):
        glob = per_core[c]["glob"].reshape(-1)
        oc = np.ascontiguousarray(res.results[c]["outT"].T)
        m = glob >= 0
        result[glob[m]] = oc[m]
    return result, res


def kernel(**inputs):
    return _run(inputs, trace=False)[0]



# revision 4
# speedup vs baseline: 1.0430x; 1.0430x over previous
"""GCNII layer on 8 TRN2 NeuronCores (Bass/Tile).

Strategy: partition nodes (and their incoming edges, bucketed by dst) across
the 8 cores; replicate the feature table (bf16) in every core's DRAM.  Per
core, nodes are greedily packed into chunks of 32 output slots balancing the
per-(chunk, src-subrange) edge counts so each (chunk,sub) fits ONE 128-edge
gather tile (TR=1).  The feature table is split into 4 sub-tables of <32768
rows so dma_gather's int16 indices reach every row; gathers run one call per
(16-chunk group, subrange) cycling the 4 SWDGE queues.

Aggregation runs TRANSPOSED: the raw gathered rows are the PE stationary
operand (bf16, FWL) and a device-generated bf16 SCALE-VALUED one-hot is the
moving operand -- oh[p, s] = 0.9*rsqrt(deg[src_p]*deg[dst_s]) iff edge p
lands in slot s -- so a single matmul per (chunk, sub) applies the edge
weight AND scatters, accumulating h3^T[d, slots] in PSUM with no separate
per-edge fold pass and no host one-hot upload.  The one-hot is built by two
narrow DVE passes per gather call (is_equal against an iota constant, then
an in-place broadcast multiply by the per-edge weight, itself computed on
device from the shipped integer degree products).  The 0.1*initial_features
residual joins each PSUM accumulation via a 0.1*I matmul; the
identity-mapped linear combine is a single wide (I + W^T) bf16 matmul with
fused ReLU(0.5*x), written out as bf16 [D, slots] blocks and un-permuted on
the host.  Host-side work is integer bucketing/layout; all per-edge float
math runs on device.
"""

import sys

if "/opt/trn_rl_repo" not in sys.path:
    sys.path.insert(0, "/opt/trn_rl_repo")

from contextlib import ExitStack

import ml_dtypes
import numpy as np

N, E, D, NC = 100000, 1600000, 128, 8
NPC = N // NC            # nodes per core: 12500
ALPHA, BETA = 0.1, 0.5
NSUB = 4                 # feature-table subranges (int16 index limit)
SR = 25000               # rows per subrange
SLOT = 32                # output slots per chunk (one-hot width)
GC = 16                  # chunks per gather group == psum block (512 slots)

F32 = np.float32
BF16 = ml_dtypes.bfloat16


def _wrap_idx(seq):
    """dma_gather index layout: i -> [i % 16, i // 16], replicated to 128
    partitions (one copy per Q7 core)."""
    blk = seq.reshape(-1, 16).T
    return np.tile(blk, (8, 1))


def _balance_nodes(deg_sub, chunks):
    """Greedy 4-dim balancing: assign nodes (rows of deg_sub [n,4]) to
    `chunks` bins (<=SLOT nodes each) minimizing max per-(bin,sub) load."""
    n = deg_sub.shape[0]
    order = np.argsort(-deg_sub.sum(1), kind="stable")
    loads = np.zeros((chunks, NSUB), np.int64)
    counts = np.zeros(chunks, np.int64)
    chunk_of = np.empty(n, np.int64)
    slot_of = np.empty(n, np.int64)
    for i in order:
        score = np.max(loads + deg_sub[i], axis=1)
        score[counts >= SLOT] = 1 << 60
        c = int(np.argmin(score))
        chunk_of[i] = c
        slot_of[i] = counts[c]
        counts[c] += 1
        loads[c] += deg_sub[i]
    return chunk_of, slot_of, loads


def _host_prep(features, initial_features, W, src, dst):
    """Integer bucketing/layout prep -> per-core device arrays."""
    src = np.ascontiguousarray(src).astype(np.int64, copy=False)
    dst = np.ascontiguousarray(dst).astype(np.int64, copy=False)
    deg = np.bincount(dst, minlength=N)
    degc = np.maximum(deg, 1).astype(np.int64)
    core_of = dst // NPC

    # smallest chunk count (multiple of GC) where every (chunk, sub) cell
    # fits one 128-edge tile on every core
    cores_tmp = None
    CHUNKS = 416
    while True:
        tmp = []
        ok = True
        for c in range(NC):
            em = core_of == c
            e_src = src[em]
            e_loc = dst[em] - c * NPC
            e_sub = e_src // SR
            deg_sub = np.zeros((NPC, NSUB), np.int64)
            np.add.at(deg_sub, (e_loc, e_sub), 1)
            chunk_of, slot_of, loads = _balance_nodes(deg_sub, CHUNKS)
            if int(loads.max()) > 128 or (CHUNKS * SLOT < NPC):
                ok = False
                break
            tmp.append((e_src, e_loc, e_sub, chunk_of, slot_of))
        if ok:
            cores_tmp = tmp
            break
        CHUNKS += GC

    cap = 128
    COLS = CHUNKS * NSUB                 # one gather tile per (chunk, sub)
    NG = CHUNKS // GC
    per_core = []
    for c in range(NC):
        e_src, e_loc, e_sub, chunk_of, slot_of = cores_tmp[c]
        e_chunk = chunk_of[e_loc]
        e_slot = slot_of[e_loc]
        o = np.lexsort((e_src, e_sub, e_chunk))
        e_src, e_slot, e_chunk, e_sub = e_src[o], e_slot[o], e_chunk[o], e_sub[o]
        e_ddeg = degc[e_loc[o] + c * NPC]    # deg[dst] per (sorted) edge
        e_sdeg = degc[e_src]
        cnt = np.bincount(e_chunk * NSUB + e_sub, minlength=CHUNKS * NSUB)
        # [CHUNKS, NSUB, cap] per-(chunk,subrange) padded tiles
        idx_arr = np.zeros((CHUNKS, NSUB, cap), np.int16)
        slt_arr = np.full((CHUNKS, NSUB, cap), -1.0, F32)  # dst slot or -1
        dsd_arr = np.ones((CHUNKS, NSUB, cap), F32)   # deg[src]*deg[dst]
        starts = np.zeros(CHUNKS * NSUB, np.int64)
        starts[1:] = np.cumsum(cnt)[:-1]
        pos = np.arange(len(e_src)) - starts[e_chunk * NSUB + e_sub]
        idx_arr[e_chunk, e_sub, pos] = (e_src - e_sub * SR).astype(np.int16)
        slt_arr[e_chunk, e_sub, pos] = e_slot
        dsd_arr[e_chunk, e_sub, pos] = (e_sdeg * e_ddeg).astype(F32)
        # gather-call order: (group g, sub r, chunk-local cl, part p)
        idx_g = idx_arr.reshape(NG, GC, NSUB, cap)
        idx_flat = idx_g.transpose(0, 2, 1, 3).reshape(-1)
        idx_dev = _wrap_idx(idx_flat).astype(np.int16)   # [128, COLS*8]
        dsd_g = dsd_arr.reshape(NG, GC, NSUB, cap).transpose(0, 2, 1, 3)
        dsd_dev = np.ascontiguousarray(dsd_g.reshape(COLS, 128).T)
        slt_g = slt_arr.reshape(NG, GC, NSUB, cap).transpose(0, 2, 1, 3)
        slt_dev = np.ascontiguousarray(slt_g.reshape(COLS, 128).T).astype(BF16)

        # node layout tables
        nodelist = np.full((CHUNKS, SLOT), -1, np.int64)
        nodelist[chunk_of, slot_of] = np.arange(NPC)
        glob = np.where(nodelist >= 0, nodelist + c * NPC, -1)
        # initpT blocked: [NG*128, GC*SLOT] bf16; block g rows = feature d,
        # cols = (chunk-local cl, slot s); value = init[glob[g*GC+cl, s], d]
        gv = glob.reshape(NG, GC * SLOT)
        initpT = np.zeros((NG, D, GC * SLOT), F32)
        m = gv >= 0
        for bi in range(NG):
            mb = m[bi]
            initpT[bi][:, mb] = initial_features[gv[bi][mb]].T
        initpT = initpT.reshape(NG * D, GC * SLOT).astype(BF16)
        per_core.append(
            dict(
                eidx=np.ascontiguousarray(idx_dev),
                edsd=dsd_dev,
                eslot=slt_dev,
                initpT=np.ascontiguousarray(initpT),
                glob=glob,
            )
        )
    return per_core, CHUNKS


_BUILD_CACHE = {}


def _build(chunks, n_rows=N, nsub=NSUB, sr=SR):
    key = (chunks, n_rows, nsub, sr)
    if key in _BUILD_CACHE:
        return _BUILD_CACHE[key]
    import concourse.bacc as bacc
    import concourse.bass as bass  # noqa: F401
    import concourse.mybir as mybir
    import concourse.tile as tile

    f32 = mybir.dt.float32
    bf16 = mybir.dt.bfloat16
    i16 = mybir.dt.int16
    Alu = mybir.AluOpType
    Act = mybir.ActivationFunctionType

    COLS = chunks * nsub             # total gather tiles (one per chunk,sub)
    IDXC = COLS * 8                  # idx cols (int16, 16-wrap => /16*128)
    NG = chunks // GC                # gather groups == psum blocks
    NIC = GC * 128                   # idxs per gather call
    ICC = NIC // 16                  # idx cols per gather call
    BW = GC * SLOT                   # psum block width (512)

    nc = bacc.Bacc("TRN2", target_bir_lowering=False, num_swdge_queues=4)
    feats = nc.dram_tensor("feats", [n_rows, D], bf16, kind="ExternalInput")
    wt = nc.dram_tensor("wt", [D, D], f32, kind="ExternalInput")
    ident = nc.dram_tensor("ident", [128, 128], f32, kind="ExternalInput")
    iotar = nc.dram_tensor("iotar", [128, GC * SLOT], bf16,
                           kind="ExternalInput")
    eidx = nc.dram_tensor("eidx", [128, IDXC], i16, kind="ExternalInput")
    edsd = nc.dram_tensor("edsd", [128, COLS], f32, kind="ExternalInput")
    eslot = nc.dram_tensor("eslot", [128, COLS], bf16, kind="ExternalInput")
    initpT = nc.dram_tensor("initpT", [NG * 128, BW], bf16,
                            kind="ExternalInput")
    outB = nc.dram_tensor("outB", [NG * 128, BW], bf16,
                          kind="ExternalOutput")

    with tile.TileContext(nc) as tc, ExitStack() as ctx:
        const = ctx.enter_context(tc.tile_pool(name="const", bufs=1))
        gpools = [ctx.enter_context(tc.tile_pool(name=f"g{r}", bufs=3))
                  for r in range(nsub)]
        opools = [ctx.enter_context(tc.tile_pool(name=f"o{r}", bufs=3))
                  for r in range(nsub)]
        ipool = ctx.enter_context(tc.tile_pool(name="init", bufs=3))
        hpool = ctx.enter_context(tc.tile_pool(name="h3", bufs=3))
        obpool = ctx.enter_context(tc.tile_pool(name="ob", bufs=3))
        ps_agg = ctx.enter_context(tc.tile_pool(name="psagg", bufs=3,
                                                space="PSUM"))
        ps_mm = ctx.enter_context(tc.tile_pool(name="psmm", bufs=2,
                                               space="PSUM"))

        wt_sb = const.tile([128, 128], f32)
        nc.sync.dma_start(out=wt_sb[:], in_=wt[:])
        id_sb = const.tile([128, 128], f32)
        nc.sync.dma_start(out=id_sb[:], in_=ident[:])
        iota_sb = const.tile([128, GC * SLOT], bf16)
        nc.sync.dma_start(out=iota_sb[:], in_=iotar[:])
        idx_sb = const.tile([128, IDXC], i16)
        nc.sync.dma_start(out=idx_sb[:], in_=eidx[:])
        dsd_sb = const.tile([128, COLS], f32)
        nc.sync.dma_start(out=dsd_sb[:], in_=edsd[:])
        slot_sb = const.tile([128, COLS], bf16)
        nc.sync.dma_start(out=slot_sb[:], in_=eslot[:])

        # W1 = I + W^T (bf16), id01 = 0.1*I (bf16)
        w1_sb = const.tile([128, 128], bf16)
        nc.vector.tensor_tensor(w1_sb[:], wt_sb[:], id_sb[:], Alu.add)
        id01_sb = const.tile([128, 128], bf16)
        nc.vector.tensor_scalar(id01_sb[:], id_sb[:], ALPHA, None, Alu.mult)

        # scl = 0.9 * rsqrt(deg[src]*deg[dst]) = 1 / sqrt(x / 0.81)
        sclf_sb = const.tile([128, COLS], f32)
        nc.scalar.activation(sclf_sb[:], dsd_sb[:], Act.Sqrt,
                             scale=float(1.0 / ((1.0 - ALPHA) ** 2)))
        nc.vector.reciprocal(sclf_sb[:], sclf_sb[:])
        scl_sb = const.tile([128, COLS], bf16)
        nc.scalar.activation(scl_sb[:], sclf_sb[:], Act.Copy)

        def issue_gathers(g):
            bufs = []
            for r in range(nsub):
                call = g * nsub + r
                lo = r * sr
                hi = min(n_rows, (r + 1) * sr)
                bufr = gpools[r].tile([128, NIC * D // 128], bf16,
                                      name=f"b{r}")
                nc.gpsimd.dma_gather(
                    out_ap=bufr[:].rearrange("p (t d) -> p t d", t=GC),
                    in_ap=feats[lo:hi, :],
                    idxs_ap=idx_sb[:, call * ICC:(call + 1) * ICC],
                    num_idxs=NIC,
                    num_idxs_reg=NIC,
                    elem_size=D,
                    single_packet=False,
                    queue_num=call % 4,
                )
                bufs.append(bufr)
            return bufs

        def issue_ohgen(g):
            ohs = []
            for r in range(nsub):
                call = g * nsub + r
                # oh[p, cl, s] = scl[p, cl] if slot[p, cl] == s else 0
                ohr = opools[r].tile([128, GC * SLOT], bf16, name=f"oh{r}")
                nc.vector.tensor_tensor(
                    ohr[:].rearrange("p (t s) -> p t s", t=GC),
                    slot_sb[:, call * GC:(call + 1) * GC]
                    .unsqueeze(-1).broadcast_to([128, GC, SLOT]),
                    iota_sb[:].rearrange("p (t s) -> p t s", t=GC),
                    Alu.is_equal)
                nc.vector.tensor_tensor(
                    ohr[:].rearrange("p (t s) -> p t s", t=GC),
                    ohr[:].rearrange("p (t s) -> p t s", t=GC),
                    scl_sb[:, call * GC:(call + 1) * GC]
                    .unsqueeze(-1).broadcast_to([128, GC, SLOT]),
                    Alu.mult)
                ohs.append(ohr)
            return ohs

        bufs = issue_gathers(0)
        for g in range(NG):
            ohs = issue_ohgen(g)
            nbufs = issue_gathers(g + 1) if g + 1 < NG else None
            itile = ipool.tile([128, BW], bf16, name="itile")
            nc.sync.dma_start(out=itile[:],
                              in_=initpT[g * 128:(g + 1) * 128, :])
            psw = ps_agg.tile([128, BW], f32, space="PSUM", name="psw")
            for cl in range(GC):
                for r in range(nsub):
                    nc.tensor.matmul(
                        psw[:, cl * SLOT:(cl + 1) * SLOT],
                        lhsT=bufs[r][:, cl * 128:(cl + 1) * 128],
                        rhs=ohs[r][:, cl * SLOT:(cl + 1) * SLOT],
                        start=(r == 0),
                        stop=False,
                    )
                # close the accumulation with the 0.1*init residual
                nc.tensor.matmul(
                    psw[:, cl * SLOT:(cl + 1) * SLOT],
                    lhsT=id01_sb[:],
                    rhs=itile[:, cl * SLOT:(cl + 1) * SLOT],
                    start=False, stop=True,
                )
            # epilogue: out = relu(0.5 * (I + W^T) @ h3T)
            h3 = hpool.tile([128, BW], bf16, tag="h3")
            nc.scalar.activation(h3[:], psw[:], Act.Copy)
            pmm = ps_mm.tile([128, BW], f32, space="PSUM", name="pmm")
            nc.tensor.matmul(pmm[:], lhsT=w1_sb[:], rhs=h3[:],
                             start=True, stop=True)
            obw = obpool.tile([128, BW], bf16)
            nc.scalar.activation(obw[:], pmm[:], Act.Relu, scale=BETA)
            nc.sync.dma_start(out=outB[g * 128:(g + 1) * 128, :], in_=obw[:])
            bufs = nbufs

    nc.compile()
    _BUILD_CACHE[key] = nc
    return nc


def _install_ntff_shim():
    """antenv.axon_hooks is absent in this image; shim it and wire the real
    NTFF profiling hook via ctypes so trace=True works under axon."""
    import contextlib
    import ctypes
    import types

    try:
        from antenv import axon_hooks  # noqa: F401
        return
    except ImportError:
        pass
    import antenv

    mod = types.ModuleType("antenv.axon_hooks")
    _hook = [None]
    mod.set_axon_ntff_profile_hook = lambda h: _hook.__setitem__(0, h)
    mod.get_axon_ntff_profile_hook = lambda: _hook[0]
    sys.modules["antenv.axon_hooks"] = mod
    antenv.axon_hooks = mod
    try:
        lib = ctypes.CDLL("/opt/axon/libaxon_pjrt.so")
    except OSError:
        return
    if not hasattr(lib, "axon_start_nrt_profile"):
        return
    lib.axon_start_nrt_profile.argtypes = [
        ctypes.POINTER(ctypes.c_int64),
        ctypes.c_size_t,
    ]
    lib.axon_start_nrt_profile.restype = ctypes.c_int64
    lib.axon_stop_nrt_profile.argtypes = [ctypes.c_char_p]
    lib.axon_stop_nrt_profile.restype = ctypes.c_int64

    @contextlib.contextmanager
    def _hook_cm(output_dir, device_ids):
        import jax

        jax.devices()
        if device_ids:
            ids = (ctypes.c_int64 * len(device_ids))(*device_ids)
            rc = lib.axon_start_nrt_profile(ids, len(device_ids))
        else:
            rc = lib.axon_start_nrt_profile(None, 0)
        if rc != 0:
            raise RuntimeError(f"axon_start_nrt_profile rc={rc}")
        try:
            yield
        finally:
            rc = lib.axon_stop_nrt_profile(output_dir.encode())
            if rc != 0:
                print(f"WARNING: axon_stop_nrt_profile rc={rc}", flush=True)

    mod.set_axon_ntff_profile_hook(_hook_cm)


def _run(inputs, trace=False, trace_cores=None):
    from concourse import bass_utils

    if trace:
        _install_ntff_shim()
    features = np.ascontiguousarray(np.asarray(inputs["features"], dtype=F32))
    initial_features = np.ascontiguousarray(
        np.asarray(inputs["initial_features"], dtype=F32)
    )
    W = np.asarray(inputs["W"], dtype=F32)
    src = np.asarray(inputs["src"])
    dst = np.asarray(inputs["dst"])
    per_core, CHUNKS = _host_prep(features, initial_features, W, src, dst)
    nc = _build(CHUNKS)
    feats_bf = np.ascontiguousarray(features.astype(BF16))
    wt_np = np.ascontiguousarray(W.T)
    ident_np = np.eye(128, dtype=F32)
    iota_np = np.ascontiguousarray(
        np.tile(np.arange(SLOT, dtype=F32)[None, :], (128, GC)).astype(BF16))
    in_maps = []
    for c in range(NC):
        pc = per_core[c]
        in_maps.append(
            dict(
                feats=feats_bf,
                wt=wt_np,
                ident=ident_np,
                iotar=iota_np,
                eidx=pc["eidx"],
                edsd=pc["edsd"],
                eslot=pc["eslot"],
                initpT=pc["initpT"],
            )
        )
    res = bass_utils.run_bass_kernel_spmd(
        nc,
        in_maps,
        core_ids=list(range(NC)),
        trace=trace,
        trace_cores=trace_cores,
    )
    NG = CHUNKS // GC
    result = np.empty((N, D), F32)
    for c in range(NC):
        glob = per_core[c]["glob"].reshape(-1)
        ob = np.asarray(res.results[c]["outB"], dtype=F32)
        # outB [NG*128, GC*SLOT]: block g rows=d, cols=(cl, s)
        oc = ob.reshape(NG, D, GC * SLOT).transpose(0, 2, 1).reshape(-1, D)
        m = glob >= 0
        result[glob[m]] = oc[m]
    return result, res


def kernel(**inputs):
    return _run(inputs, trace=False)[0]
